# revision 10
# baseline (speedup 1.0000x reference)
"""nn_CSRSparseRetrievalModel: CSR sparse retrieval (SPLADE-style top-k)
as a Bass/Tile kernel for Trainium2, sharded across 8 NeuronCores.

Strategy (inverted index): the host builds a query-INDEPENDENT CSC
("inverted index") layout of the document matrix, sharded by document
across the 8 cores: for every vocab term v and core c, a fixed-capacity
column holding (doc%128, doc//128, value) of the documents containing v.
Per query, each core:
  1. dma_gather's the 32 query-term columns (transposed so entries land
     one-per-partition),
  2. scales values by the query weights,
  3. scatters entries into a [128, 512] score grid via one-hot matmuls
     accumulated in PSUM: S[doc%128, doc//128] += val*qv  (weights =
     onehot(doc%128), moving = val*qv*onehot(doc//128)),
  4. writes the 62.5K-doc score grid back to DRAM.
The host assembles the 500K approximate scores, picks top candidates and
rescores them exactly in fp32 (same candidate-rescue as the baseline).

kernel(**inputs) -> (top_values float32 [k], top_indices int32 [k])
"""

import numpy as np

import concourse.bass as bass
import concourse.tile as tile
from concourse import bacc, mybir
from concourse.alu_op_type import AluOpType

VOCAB = 30522
N_CORES = 8
CAP = 384           # default entries per (core, vocab) column (auto-grown)
QPAD = 128          # dma_gather transpose needs num_idxs % 128 == 0
NHI = 512           # score grid free dim (>= ceil(62500/128) = 489)


# ---------------------------------------------------------------------------
# Host-side planning
# ---------------------------------------------------------------------------

class Plan:
    pass


def make_plan(crow, q_indices, q_values, vocab, n_cores=8, n_sec=5):
    """Query-side prep: padded query index table (gather idxs wrapped
    layout), replicated query values, iota constants."""
    p = Plan()
    n_docs = crow.shape[0] - 1
    assert n_docs % n_cores == 0
    p.n_cores = n_cores
    p.n_docs = n_docs
    p.dpc = n_docs // n_cores
    assert p.dpc <= 128 * NHI

    qi = np.asarray(q_indices).reshape(-1).astype(np.int64)
    qv = np.asarray(q_values).reshape(-1).astype(np.float64)
    p.n_q = len(qi)
    assert p.n_q <= QPAD

    # dense query for host-side exact rescoring (duplicates coalesced)
    dense = np.zeros(vocab, dtype=np.float64)
    np.add.at(dense, qi, qv)
    p.query_dense = dense.astype(np.float32)
    nz = np.nonzero(dense)[0]
    p.terms = [(int(i), float(np.float32(dense[i]))) for i in nz]

    # gather idx table: idx i lives at [p%16 == i%16, i//16], replicated
    qidx_pad = np.zeros(QPAD, dtype=np.int16)
    qidx_pad[: p.n_q] = qi.astype(np.int16)
    wrapped = qidx_pad.reshape(QPAD // 16, 16).T  # [16, QPAD//16]
    p.qidx_sb = np.tile(wrapped, (8, 1))  # [128, QPAD//16]

    p.qv_f32 = qv.astype(np.float32)
    p.iota_o = np.broadcast_to(
        np.arange(128, dtype=np.float16)[None, :], (128, 128)).copy()
    p.iota_h = np.broadcast_to(
        np.arange(NHI, dtype=np.float16)[None, :], (128, NHI)).copy()
    p.cap = CAP
    p.repeat = 1
    return p


_CSC_CACHE = {}


def build_csc(crow, indice, values, n_cores=N_CORES, vocab=VOCAB):
    """Query-independent inverted index, sharded by document across cores.

    Returns (csc, cap): int16 array [n_cores, vocab, 3*cap]: per column,
    three planes of `cap` int16s: doc%128, doc//128, value (bf16 bit
    pattern). Unused slots are zero (lo=hi=0, val=+0.0 -> no contribution).
    """
    import ml_dtypes
    crow = np.asarray(crow, dtype=np.int64)
    ind = np.asarray(indice, dtype=np.int32)
    val = np.asarray(values, dtype=np.float32)
    key_fp = (crow.shape[0], ind.shape[0], int(crow[-1]),
              int(ind[:1000].sum()), float(val[:1000].sum()))
    if key_fp in _CSC_CACHE:
        return _CSC_CACHE[key_fp]

    n_docs = crow.shape[0] - 1
    dpc = n_docs // n_cores
    # per-doc local fields, expanded to per-entry (entries are doc-major in
    # CSR order, so each core's entries are one contiguous CSR range)
    dloc_doc = (np.arange(n_docs, dtype=np.int32) % dpc)
    lengths = np.diff(crow)
    lo_rep = np.repeat((dloc_doc % 128).astype(np.int16), lengths)
    hi_rep = np.repeat((dloc_doc // 128).astype(np.int16), lengths)

    core_data = []
    max_count = 0
    for c in range(n_cores):
        s, e = int(crow[c * dpc]), int(crow[(c + 1) * dpc])
        ind_c = ind[s:e]
        order = np.argsort(ind_c, kind="stable")
        ind_cs = ind_c[order].astype(np.int32)
        counts = np.bincount(ind_cs, minlength=vocab)
        max_count = max(max_count, int(counts.max()))
        core_data.append((s, e, order, ind_cs, counts))

    cap = max(CAP, -(-max_count // 128) * 128)
    csc = np.zeros((n_cores, vocab, 3 * cap), dtype=np.int16)
    for c, (s, e, order, ind_cs, counts) in enumerate(core_data):
        starts = np.zeros_like(counts)
        np.cumsum(counts[:-1], out=starts[1:])
        m = e - s
        rank = np.arange(m, dtype=np.int32) - np.repeat(starts, counts).astype(np.int32)
        pos = ind_cs * (3 * cap) + rank
        flat = csc[c].reshape(-1)
        flat[pos] = lo_rep[s:e][order]
        flat[pos + cap] = hi_rep[s:e][order]
        flat[pos + 2 * cap] = val[s:e][order].astype(ml_dtypes.bfloat16).view(np.int16)
    _CSC_CACHE.clear()
    _CSC_CACHE[key_fp] = (csc, cap)
    return csc, cap


def make_core_inputs(p, crow, indice, values):
    """Per-core input dicts for the SPMD run. Sets p.cap/p.qv_sb."""
    import ml_dtypes
    csc, cap = build_csc(crow, indice, values, n_cores=p.n_cores)
    p.cap = cap
    nslot = cap // 128
    p.qv_sb = np.broadcast_to(
        p.qv_f32.astype(ml_dtypes.bfloat16)[None, None, :],
        (128, nslot, p.n_q)).copy()
    ins = []
    for c in range(p.n_cores):
        ins.append(dict(
            csc=csc[c],
            qidx=p.qidx_sb,
            qv=p.qv_sb,
            iota_o=p.iota_o,
            iota_h=p.iota_h,
        ))
    return ins


# ---------------------------------------------------------------------------
# Device program
# ---------------------------------------------------------------------------

def build_program(p, n_devices=8):
    nc = bacc.Bacc("TRN2", target_bir_lowering=False, debug=False,
                   num_devices=n_devices)
    f32, bf16, fp16 = mybir.dt.float32, mybir.dt.bfloat16, mybir.dt.float16
    i16 = mybir.dt.int16
    cap = p.cap
    NSLOT = cap // 128

    csc = nc.declare_dram_parameter("csc", [VOCAB, 3 * cap], i16, isOutput=False)
    qidx = nc.declare_dram_parameter("qidx", [128, QPAD // 16], i16, isOutput=False)
    qvp = nc.declare_dram_parameter("qv", [128, NSLOT, p.n_q], bf16, isOutput=False)
    iota_o = nc.declare_dram_parameter("iota_o", [128, 128], fp16, isOutput=False)
    iota_h = nc.declare_dram_parameter("iota_h", [128, NHI], fp16, isOutput=False)
    scores_out = nc.declare_dram_parameter("scores", [128, NHI], f32, isOutput=True)

    n_batch = NSLOT * p.n_q

    with tile.TileContext(nc) as tc:
        with tc.tile_pool(name="const", bufs=1) as cpool, \
             tc.tile_pool(name="work", bufs=2) as wpool, \
             tc.tile_pool(name="onehot", bufs=4) as opool, \
             tc.tile_pool(name="ps", bufs=1, space=bass.MemorySpace.PSUM) as ppool:

            qidx_t = cpool.tile([128, QPAD // 16], i16, tag="qidx")
            nc.sync.dma_start(qidx_t[:], qidx[:])
            qv_t = cpool.tile([128, NSLOT, p.n_q], bf16, tag="qv")
            nc.sync.dma_start(qv_t[:], qvp[:])
            io_t = cpool.tile([128, 128], fp16, tag="iota_o")
            nc.sync.dma_start(io_t[:], iota_o[:])
            ih_t = cpool.tile([128, NHI], fp16, tag="iota_h")
            nc.sync.dma_start(ih_t[:], iota_h[:])

            for _rep in range(p.repeat):
                # 1) gather the query columns, transposed: entry e of
                # column t, field plane f -> g[e%128, f*NSLOT + e//128, t]
                g = wpool.tile([128, 3 * NSLOT, QPAD], i16, tag="g")
                nc.gpsimd.dma_gather(
                    g[:], csc[:, :], qidx_t[:],
                    num_idxs=QPAD, num_idxs_reg=QPAD,
                    elem_size=3 * cap, transpose=True)

                # 2) unpack fields for the used query slots
                lo_f = wpool.tile([128, NSLOT, p.n_q], f32, tag="lo")
                nc.vector.tensor_copy(lo_f[:], g[:, 0:NSLOT, 0:p.n_q])
                hi_f = wpool.tile([128, NSLOT, p.n_q], f32, tag="hi")
                nc.vector.tensor_copy(hi_f[:], g[:, NSLOT:2 * NSLOT, 0:p.n_q])
                sval = wpool.tile([128, NSLOT, p.n_q], f32, tag="sval")
                nc.vector.tensor_tensor(
                    sval[:],
                    g[:, 2 * NSLOT:3 * NSLOT, 0:p.n_q].bitcast(bf16),
                    qv_t[:], AluOpType.mult)

                # 3) one-hot scatter via PSUM-accumulated matmuls
                ps = ppool.tile([128, NHI], f32, tag="ps")
                for b in range(n_batch):
                    t, s = divmod(b, NSLOT)
                    w_t = opool.tile([128, 128], bf16, tag="w")
                    nc.vector.tensor_scalar(
                        w_t[:], io_t[:], lo_f[:, s, t:t + 1], None,
                        AluOpType.is_equal)
                    x_t = opool.tile([128, NHI], bf16, tag="x")
                    nc.vector.tensor_scalar(
                        x_t[:], ih_t[:], hi_f[:, s, t:t + 1],
                        sval[:, s, t:t + 1],
                        AluOpType.is_equal, AluOpType.mult)
                    nc.tensor.matmul(
                        ps[:], w_t[:], x_t[:],
                        start=(b == 0), stop=(b == n_batch - 1))

                # 4) write back the score grid
                out_sb = wpool.tile([128, NHI], f32, tag="out")
                nc.vector.tensor_copy(out_sb[:], ps[:])
                nc.sync.dma_start(scores_out[:], out_sb[:])

    nc.compile()
    return nc


# ---------------------------------------------------------------------------
# Host-side postprocessing
# ---------------------------------------------------------------------------

def scores_from_results(p, results):
    """results: per-core dicts with 'scores' [128, NHI]; local doc d is at
    [d % 128, d // 128]."""
    all_scores = np.zeros(p.n_docs, dtype=np.float32)
    for c in range(p.n_cores):
        sc = np.asarray(results[c]["scores"])  # [128, NHI]
        flat = sc.T.reshape(-1)[: p.dpc]
        all_scores[c * p.dpc:(c + 1) * p.dpc] = flat
    return all_scores


def exact_topk(p, approx_scores, crow, indice, values, top_k, n_cand=4096):
    """Pick candidates by approximate score, rescore exactly, return top_k."""
    crow = np.asarray(crow)
    indice = np.asarray(indice)
    values = np.asarray(values)
    n_cand = min(n_cand, p.n_docs)
    cand = np.argpartition(-approx_scores, n_cand - 1)[:n_cand]
    qd = p.query_dense
    exact = np.empty(n_cand, dtype=np.float32)
    for i, d in enumerate(cand):
        s, e = int(crow[d]), int(crow[d + 1])
        exact[i] = np.float32(
            np.sum(values[s:e].astype(np.float32) * qd[indice[s:e]],
                   dtype=np.float32))
    order = np.lexsort((cand, -exact.astype(np.float64)))
    top = order[:top_k]
    return exact[top].astype(np.float32), cand[top].astype(np.int32)


# ---------------------------------------------------------------------------
# SPMD execution via PJRT (axon) with repeat timing
# ---------------------------------------------------------------------------

def run_spmd_timed(nc, in_maps, n_cores=8, n_iters=3):
    """Mirror bass2jax.run_bass_via_pjrt but jit once and time each call.

    Returns (results, times_s): results like run_bass_kernel_spmd
    (list per core of {name: np.ndarray}), times_s = wall time per call.
    """
    import time
    import jax
    from jax.sharding import Mesh, PartitionSpec
    from jax.experimental.shard_map import shard_map
    from concourse import bass2jax, mybir as mb

    bass2jax.install_neuronx_cc_hook()
    assert nc.dbg_addr is None or not nc.dbg_callbacks

    partition_name = nc.partition_id_tensor.name if nc.partition_id_tensor else None
    in_names, out_names, out_avals, zero_outs = [], [], [], []
    for alloc in nc.m.functions[0].allocations:
        if not isinstance(alloc, mb.MemoryLocationSet):
            continue
        name = alloc.memorylocations[0].name
        if alloc.kind == "ExternalInput":
            if name != partition_name:
                in_names.append(name)
        elif alloc.kind == "ExternalOutput":
            shape = tuple(alloc.tensor_shape)
            dtype = mb.dt.np(alloc.dtype)
            out_names.append(name)
            out_avals.append(jax.core.ShapedArray(shape, dtype))
            zero_outs.append(np.zeros(shape, dtype))
    n_params = len(in_names)
    n_outs = len(out_avals)
    in_names_all = in_names + out_names
    if partition_name is not None:
        in_names_all = in_names_all + [partition_name]

    donate = tuple(range(n_params, n_params + n_outs))

    def _body(*args):
        operands = list(args)
        if partition_name is not None:
            operands.append(bass2jax.partition_id_tensor())
        outs = bass2jax._bass_exec_p.bind(
            *operands,
            out_avals=tuple(out_avals),
            in_names=tuple(in_names_all),
            out_names=tuple(out_names),
            lowering_input_output_aliases=(),
            sim_require_finite=True,
            sim_require_nnan=True,
            nc=nc,
        )
        return tuple(outs)

    devices = jax.devices()[:n_cores]
    mesh = Mesh(np.asarray(devices), ("core",))
    in_specs = (PartitionSpec("core"),) * (n_params + n_outs)
    out_specs = (PartitionSpec("core"),) * n_outs
    sharded = jax.jit(
        shard_map(_body, mesh=mesh, in_specs=in_specs, out_specs=out_specs,
                  check_rep=False),
        donate_argnums=donate, keep_unused=True)

    from jax.sharding import NamedSharding
    shd = NamedSharding(mesh, PartitionSpec("core"))
    concat_in = [
        jax.device_put(
            np.concatenate([np.asarray(in_maps[c][name]) for c in range(n_cores)],
                           axis=0), shd)
        for name in in_names
    ]
    jax.block_until_ready(concat_in)
    times = []
    out_arrs = None
    for it in range(n_iters):
        concat_zeros = [
            jax.device_put(
                np.zeros((n_cores * z.shape[0], *z.shape[1:]), z.dtype), shd)
            for z in zero_outs
        ]
        jax.block_until_ready(concat_zeros)
        t0 = time.perf_counter()
        res = sharded(*concat_in, *concat_zeros)
        jax.block_until_ready(res)
        t1 = time.perf_counter()
        times.append(t1 - t0)
        out_arrs = res
    results = [
        {name: np.asarray(out_arrs[i]).reshape(n_cores, *out_avals[i].shape)[c]
         for i, name in enumerate(out_names)}
        for c in range(n_cores)
    ]
    return results, times


def kernel(q_indices, q_values, crow, indice, values, top_k, n_iters=1,
           _cache={}):
    """Full-input kernel: shard internally over 8 cores, return (vals, idx)."""
    crow_np = np.asarray(crow)
    ind_np = np.asarray(indice)
    val_np = np.asarray(values)
    p = make_plan(crow_np, np.asarray(q_indices), np.asarray(q_values), VOCAB)
    core_ins = make_core_inputs(p, crow_np, ind_np, val_np)
    key = (p.n_q, p.repeat, p.cap)
    if key in _cache:
        nc = _cache[key]
    else:
        nc = build_program(p, n_devices=8)
        _cache[key] = nc
    try:
        results, times = run_spmd_timed(nc, core_ins, n_cores=8, n_iters=n_iters)
    except Exception:
        # axon workers occasionally desync; one retry after re-jit
        import time as _time
        _time.sleep(5)
        results, times = run_spmd_timed(nc, core_ins, n_cores=8, n_iters=n_iters)
    kernel.last_times = times
    approx = scores_from_results(p, results)
    kernel.last_approx = approx
    vals, idx = exact_topk(p, approx, crow_np, ind_np, val_np, int(top_k))
    return vals, idx


# revision 12
# speedup vs baseline: 1.1450x; 1.1450x over previous
"""nn_CSRSparseRetrievalModel: CSR sparse retrieval (SPLADE-style top-k)
as a Bass/Tile kernel for Trainium2, sharded across 8 NeuronCores.

Strategy (inverted index): the host builds a query-INDEPENDENT CSC
("inverted index") layout of the document matrix, sharded by document
across the 8 cores: for every vocab term v and core c, a fixed-capacity
column holding (doc%128, doc//128, value) of the documents containing v.
Per query, each core:
  1. dma_gather's the 32 query-term columns (transposed so entries land
     one-per-partition),
  2. scales values by the query weights,
  3. scatters entries into a [128, 512] score grid via one-hot matmuls
     accumulated in PSUM: S[doc%128, doc//128] += val*qv  (weights =
     onehot(doc%128), moving = val*qv*onehot(doc//128)),
  4. writes the 62.5K-doc score grid back to DRAM.
The host assembles the 500K approximate scores, picks top candidates and
rescores them exactly in fp32 (same candidate-rescue as the baseline).

kernel(**inputs) -> (top_values float32 [k], top_indices int32 [k])
"""

import numpy as np

import concourse.bass as bass
import concourse.tile as tile
from concourse import bacc, mybir
from concourse.alu_op_type import AluOpType

VOCAB = 30522
N_CORES = 8
CAP = 384           # default entries per (core, vocab) column (auto-grown)
QPAD = 128          # dma_gather transpose needs num_idxs % 128 == 0
NHI = 512           # score grid free dim (>= ceil(62500/128) = 489)


# ---------------------------------------------------------------------------
# Host-side planning
# ---------------------------------------------------------------------------

class Plan:
    pass


def make_plan(crow, q_indices, q_values, vocab, n_cores=8, n_sec=5):
    """Query-side prep: padded query index table (gather idxs wrapped
    layout), replicated query values, iota constants."""
    p = Plan()
    n_docs = crow.shape[0] - 1
    assert n_docs % n_cores == 0
    p.n_cores = n_cores
    p.n_docs = n_docs
    p.dpc = n_docs // n_cores
    assert p.dpc <= 128 * NHI

    qi = np.asarray(q_indices).reshape(-1).astype(np.int64)
    qv = np.asarray(q_values).reshape(-1).astype(np.float64)
    p.n_q = len(qi)
    assert p.n_q <= QPAD

    # dense query for host-side exact rescoring (duplicates coalesced)
    dense = np.zeros(vocab, dtype=np.float64)
    np.add.at(dense, qi, qv)
    p.query_dense = dense.astype(np.float32)
    nz = np.nonzero(dense)[0]
    p.terms = [(int(i), float(np.float32(dense[i]))) for i in nz]

    # gather idx table: idx i lives at [p%16 == i%16, i//16], replicated
    qidx_pad = np.zeros(QPAD, dtype=np.int16)
    qidx_pad[: p.n_q] = qi.astype(np.int16)
    wrapped = qidx_pad.reshape(QPAD // 16, 16).T  # [16, QPAD//16]
    p.qidx_sb = np.tile(wrapped, (8, 1))  # [128, QPAD//16]

    p.qv_f32 = qv.astype(np.float32)
    p.iota_o = np.broadcast_to(
        np.arange(128, dtype=np.float16)[None, :], (128, 128)).copy()
    p.iota_h = np.broadcast_to(
        np.arange(NHI, dtype=np.float16)[None, :], (128, NHI)).copy()
    p.cap = CAP
    p.repeat = 1
    return p


_CSC_CACHE = {}


def build_csc(crow, indice, values, n_cores=N_CORES, vocab=VOCAB):
    """Query-independent inverted index, sharded by document across cores.

    Returns (csc, cap): int16 array [n_cores, vocab, 3*cap]: per column,
    three planes of `cap` int16s: doc%128, doc//128, value (bf16 bit
    pattern). Unused slots are zero (lo=hi=0, val=+0.0 -> no contribution).
    """
    import ml_dtypes
    crow = np.asarray(crow, dtype=np.int64)
    ind = np.asarray(indice, dtype=np.int32)
    val = np.asarray(values, dtype=np.float32)
    key_fp = (crow.shape[0], ind.shape[0], int(crow[-1]),
              int(ind[:1000].sum()), float(val[:1000].sum()))
    if key_fp in _CSC_CACHE:
        return _CSC_CACHE[key_fp]

    n_docs = crow.shape[0] - 1
    dpc = n_docs // n_cores
    # per-doc local fields, expanded to per-entry (entries are doc-major in
    # CSR order, so each core's entries are one contiguous CSR range)
    dloc_doc = (np.arange(n_docs, dtype=np.int32) % dpc)
    lengths = np.diff(crow)
    lo_rep = np.repeat((dloc_doc % 128).astype(np.int16), lengths)
    hi_rep = np.repeat((dloc_doc // 128).astype(np.int16), lengths)

    core_data = []
    max_count = 0
    for c in range(n_cores):
        s, e = int(crow[c * dpc]), int(crow[(c + 1) * dpc])
        ind_c = ind[s:e]
        order = np.argsort(ind_c, kind="stable")
        ind_cs = ind_c[order].astype(np.int32)
        counts = np.bincount(ind_cs, minlength=vocab)
        max_count = max(max_count, int(counts.max()))
        core_data.append((s, e, order, ind_cs, counts))

    cap = max(CAP, -(-max_count // 128) * 128)
    csc = np.zeros((n_cores, vocab, 3 * cap), dtype=np.int16)
    for c, (s, e, order, ind_cs, counts) in enumerate(core_data):
        starts = np.zeros_like(counts)
        np.cumsum(counts[:-1], out=starts[1:])
        m = e - s
        rank = np.arange(m, dtype=np.int32) - np.repeat(starts, counts).astype(np.int32)
        pos = ind_cs * (3 * cap) + rank
        flat = csc[c].reshape(-1)
        flat[pos] = lo_rep[s:e][order]
        flat[pos + cap] = hi_rep[s:e][order]
        flat[pos + 2 * cap] = val[s:e][order].astype(ml_dtypes.bfloat16).view(np.int16)
    _CSC_CACHE.clear()
    _CSC_CACHE[key_fp] = (csc, cap)
    return csc, cap


def make_core_inputs(p, crow, indice, values):
    """Per-core input dicts for the SPMD run. Sets p.cap/p.qv_sb."""
    import ml_dtypes
    csc, cap = build_csc(crow, indice, values, n_cores=p.n_cores)
    p.cap = cap
    nslot = cap // 128
    p.qv_sb = np.broadcast_to(
        p.qv_f32.astype(ml_dtypes.bfloat16)[None, None, :],
        (128, nslot, p.n_q)).copy()
    ins = []
    for c in range(p.n_cores):
        ins.append(dict(
            csc=csc[c],
            qidx=p.qidx_sb,
            qv=p.qv_sb,
            iota_o=p.iota_o,
            iota_h=p.iota_h,
        ))
    return ins


# ---------------------------------------------------------------------------
# Device program
# ---------------------------------------------------------------------------

def build_program(p, n_devices=8):
    nc = bacc.Bacc("TRN2", target_bir_lowering=False, debug=False,
                   num_devices=n_devices)
    f32, bf16, fp16 = mybir.dt.float32, mybir.dt.bfloat16, mybir.dt.float16
    i16 = mybir.dt.int16
    cap = p.cap
    NSLOT = cap // 128

    csc = nc.declare_dram_parameter("csc", [VOCAB, 3 * cap], i16, isOutput=False)
    qidx = nc.declare_dram_parameter("qidx", [128, QPAD // 16], i16, isOutput=False)
    qvp = nc.declare_dram_parameter("qv", [128, NSLOT, p.n_q], bf16, isOutput=False)
    iota_o = nc.declare_dram_parameter("iota_o", [128, 128], fp16, isOutput=False)
    iota_h = nc.declare_dram_parameter("iota_h", [128, NHI], fp16, isOutput=False)
    scores_out = nc.declare_dram_parameter("scores", [128, NHI], f32, isOutput=True)

    n_batch = NSLOT * p.n_q

    with tile.TileContext(nc) as tc:
        with tc.tile_pool(name="const", bufs=1) as cpool, \
             tc.tile_pool(name="work", bufs=2) as wpool, \
             tc.tile_pool(name="onehot", bufs=4) as opool, \
             tc.tile_pool(name="ps", bufs=1, space=bass.MemorySpace.PSUM) as ppool:

            qidx_t = cpool.tile([128, QPAD // 16], i16, tag="qidx")
            nc.sync.dma_start(qidx_t[:], qidx[:])
            qv_t = cpool.tile([128, NSLOT, p.n_q], bf16, tag="qv")
            nc.sync.dma_start(qv_t[:], qvp[:])
            io_t = cpool.tile([128, 128], fp16, tag="iota_o")
            nc.sync.dma_start(io_t[:], iota_o[:])
            ih_t = cpool.tile([128, NHI], fp16, tag="iota_h")
            nc.sync.dma_start(ih_t[:], iota_h[:])

            for _rep in range(p.repeat):
                # 1) gather the query columns, transposed: entry e of
                # column t, field plane f -> g[e%128, f*NSLOT + e//128, t]
                g = wpool.tile([128, 3 * NSLOT, QPAD], i16, tag="g")
                nc.gpsimd.dma_gather(
                    g[:], csc[:, :], qidx_t[:],
                    num_idxs=QPAD, num_idxs_reg=QPAD,
                    elem_size=3 * cap, transpose=True)

                # 2) unpack fields for the used query slots
                lo_f = wpool.tile([128, NSLOT, p.n_q], f32, tag="lo")
                nc.vector.tensor_copy(lo_f[:], g[:, 0:NSLOT, 0:p.n_q])
                hi_f = wpool.tile([128, NSLOT, p.n_q], f32, tag="hi")
                nc.vector.tensor_copy(hi_f[:], g[:, NSLOT:2 * NSLOT, 0:p.n_q])
                sval = wpool.tile([128, NSLOT, p.n_q], f32, tag="sval")
                nc.vector.tensor_tensor(
                    sval[:],
                    g[:, 2 * NSLOT:3 * NSLOT, 0:p.n_q].bitcast(bf16),
                    qv_t[:], AluOpType.mult)

                # 3) one-hot scatter via PSUM-accumulated matmuls
                ps = ppool.tile([128, NHI], f32, tag="ps")
                for b in range(n_batch):
                    t, s = divmod(b, NSLOT)
                    w_t = opool.tile([128, 128], bf16, tag="w")
                    nc.vector.tensor_scalar(
                        w_t[:], io_t[:], lo_f[:, s, t:t + 1], None,
                        AluOpType.is_equal)
                    x_t = opool.tile([128, NHI], bf16, tag="x")
                    nc.vector.tensor_scalar(
                        x_t[:], ih_t[:], hi_f[:, s, t:t + 1],
                        sval[:, s, t:t + 1],
                        AluOpType.is_equal, AluOpType.mult)
                    nc.tensor.matmul(
                        ps[:], w_t[:], x_t[:],
                        start=(b == 0), stop=(b == n_batch - 1))

                # 4) write back the score grid
                out_sb = wpool.tile([128, NHI], f32, tag="out")
                nc.vector.tensor_copy(out_sb[:], ps[:])
                nc.sync.dma_start(scores_out[:], out_sb[:])

    nc.compile()
    return nc


# ---------------------------------------------------------------------------
# Host-side postprocessing
# ---------------------------------------------------------------------------

def scores_from_results(p, results):
    """results: per-core dicts with 'scores' [128, NHI]; local doc d is at
    [d % 128, d // 128]."""
    all_scores = np.zeros(p.n_docs, dtype=np.float32)
    for c in range(p.n_cores):
        sc = np.asarray(results[c]["scores"])  # [128, NHI]
        flat = sc.T.reshape(-1)[: p.dpc]
        all_scores[c * p.dpc:(c + 1) * p.dpc] = flat
    return all_scores


def exact_topk(p, approx_scores, crow, indice, values, top_k, n_cand=4096):
    """Pick candidates by approximate score, rescore exactly, return top_k."""
    crow = np.asarray(crow)
    indice = np.asarray(indice)
    values = np.asarray(values)
    n_cand = min(n_cand, p.n_docs)
    cand = np.argpartition(-approx_scores, n_cand - 1)[:n_cand]
    qd = p.query_dense
    exact = np.empty(n_cand, dtype=np.float32)
    for i, d in enumerate(cand):
        s, e = int(crow[d]), int(crow[d + 1])
        exact[i] = np.float32(
            np.sum(values[s:e].astype(np.float32) * qd[indice[s:e]],
                   dtype=np.float32))
    order = np.lexsort((cand, -exact.astype(np.float64)))
    top = order[:top_k]
    return exact[top].astype(np.float32), cand[top].astype(np.int32)


# ---------------------------------------------------------------------------
# SPMD execution via PJRT (axon) with repeat timing
# ---------------------------------------------------------------------------

def make_spmd_runner(nc, in_maps, n_cores=8):
    """Build a jitted SPMD callable for `nc` with device-resident inputs.

    Returns (run_fn, out_names, out_avals): run_fn() executes once
    (blocking) and returns the output arrays.
    """
    import jax
    from jax.sharding import Mesh, PartitionSpec
    from jax.experimental.shard_map import shard_map
    from concourse import bass2jax, mybir as mb

    bass2jax.install_neuronx_cc_hook()
    assert nc.dbg_addr is None or not nc.dbg_callbacks

    partition_name = nc.partition_id_tensor.name if nc.partition_id_tensor else None
    in_names, out_names, out_avals, zero_outs = [], [], [], []
    for alloc in nc.m.functions[0].allocations:
        if not isinstance(alloc, mb.MemoryLocationSet):
            continue
        name = alloc.memorylocations[0].name
        if alloc.kind == "ExternalInput":
            if name != partition_name:
                in_names.append(name)
        elif alloc.kind == "ExternalOutput":
            shape = tuple(alloc.tensor_shape)
            dtype = mb.dt.np(alloc.dtype)
            out_names.append(name)
            out_avals.append(jax.core.ShapedArray(shape, dtype))
            zero_outs.append(np.zeros(shape, dtype))
    n_params = len(in_names)
    n_outs = len(out_avals)
    in_names_all = in_names + out_names
    if partition_name is not None:
        in_names_all = in_names_all + [partition_name]

    donate = tuple(range(n_params, n_params + n_outs))

    def _body(*args):
        operands = list(args)
        if partition_name is not None:
            operands.append(bass2jax.partition_id_tensor())
        outs = bass2jax._bass_exec_p.bind(
            *operands,
            out_avals=tuple(out_avals),
            in_names=tuple(in_names_all),
            out_names=tuple(out_names),
            lowering_input_output_aliases=(),
            sim_require_finite=True,
            sim_require_nnan=True,
            nc=nc,
        )
        return tuple(outs)

    devices = jax.devices()[:n_cores]
    mesh = Mesh(np.asarray(devices), ("core",))
    in_specs = (PartitionSpec("core"),) * (n_params + n_outs)
    out_specs = (PartitionSpec("core"),) * n_outs
    sharded = jax.jit(
        shard_map(_body, mesh=mesh, in_specs=in_specs, out_specs=out_specs,
                  check_rep=False),
        donate_argnums=donate, keep_unused=True)

    from jax.sharding import NamedSharding
    shd = NamedSharding(mesh, PartitionSpec("core"))
    concat_in = [
        jax.device_put(
            np.concatenate([np.asarray(in_maps[c][name]) for c in range(n_cores)],
                           axis=0), shd)
        for name in in_names
    ]
    jax.block_until_ready(concat_in)

    def run_fn():
        concat_zeros = [
            jax.device_put(
                np.zeros((n_cores * z.shape[0], *z.shape[1:]), z.dtype), shd)
            for z in zero_outs
        ]
        jax.block_until_ready(concat_zeros)
        res = sharded(*concat_in, *concat_zeros)
        jax.block_until_ready(res)
        return res

    return run_fn, out_names, out_avals


def run_spmd_timed(nc, in_maps, n_cores=8, n_iters=3):
    """Jit once and time each call. Returns (results, times_s): results like
    run_bass_kernel_spmd (list per core of {name: np.ndarray})."""
    import time
    run_fn, out_names, out_avals = make_spmd_runner(nc, in_maps, n_cores)
    times = []
    out_arrs = None
    for it in range(n_iters):
        t0 = time.perf_counter()
        out_arrs = run_fn()
        t1 = time.perf_counter()
        times.append(t1 - t0)
    results = [
        {name: np.asarray(out_arrs[i]).reshape(n_cores, *out_avals[i].shape)[c]
         for i, name in enumerate(out_names)}
        for c in range(n_cores)
    ]
    return results, times


def kernel(q_indices, q_values, crow, indice, values, top_k, n_iters=1,
           _cache={}):
    """Full-input kernel: shard internally over 8 cores, return (vals, idx)."""
    crow_np = np.asarray(crow)
    ind_np = np.asarray(indice)
    val_np = np.asarray(values)
    p = make_plan(crow_np, np.asarray(q_indices), np.asarray(q_values), VOCAB)
    core_ins = make_core_inputs(p, crow_np, ind_np, val_np)
    key = (p.n_q, p.repeat, p.cap)
    if key in _cache:
        nc = _cache[key]
    else:
        nc = build_program(p, n_devices=8)
        _cache[key] = nc
    try:
        results, times = run_spmd_timed(nc, core_ins, n_cores=8, n_iters=n_iters)
    except Exception:
        # axon workers occasionally desync; one retry after re-jit
        import time as _time
        _time.sleep(5)
        results, times = run_spmd_timed(nc, core_ins, n_cores=8, n_iters=n_iters)
    kernel.last_times = times
    approx = scores_from_results(p, results)
    kernel.last_approx = approx
    vals, idx = exact_topk(p, approx, crow_np, ind_np, val_np, int(top_k))
    return vals, idx
